# revision 1
# baseline (speedup 1.0000x reference)
"""Trainium2 Bass kernel for nn_AutoEncoder (scatter_memory).

Strategy (B-shard over 8 cores, 2 batch rows per core):
- Host: linearize+sort indices (replicated metadata), segment-combine
  duplicate voxels, build per-core wrapped scatter sources for the 64
  stride-512B scatter groups (int16 idx limit + 256B-stride DMA rule).
- Device per core: 64 collision-free dma_scatter_add calls (4 SWDGE
  queues) place per-voxel sums into a zero-donated HBM grid (V,2) f32;
  one strided readback puts the grid in SBUF as [z=128, 32768]; chunked
  DVE/ACT passes compute sum|d| and sum d^2 along x,y,z; per-partition
  partials go back to the host, which does the final 128-way sum+scale.
"""
import sys
import numpy as np

sys.path.insert(0, '/opt/trn_rl_repo')

N_CORES = 8
B, N, XS = 16, 1_000_000, 128
V = XS * XS * XS          # 2097152
NSEC = 64                 # v mod 64 scatter groups
NBLK = V // NSEC          # 32768 = int16-addressable elements per call
NUM_PAIRS = float(2 * XS * XS - 2 * XS)

_compiled = {}


def _host_prep(values, indices):
    idx = indices.astype(np.int64)
    lin = (idx[:, 0] * XS + idx[:, 1]) * XS + idx[:, 2]
    order = np.argsort(lin, kind='stable')
    lin_s = lin[order]
    uniq, starts = np.unique(lin_s, return_index=True)
    vals_s = values[:, order]
    sums = np.add.reduceat(vals_s, starts, axis=1)      # (16, U)

    o = (uniq % NSEC).astype(np.int64)
    blk = (uniq // NSEC).astype(np.int64)
    sec_order = np.lexsort((uniq, o))
    o_sorted = o[sec_order]
    blk_sorted = blk[sec_order].astype(np.int16)
    sums_sorted = sums[:, sec_order]                    # (16, U)

    counts = np.bincount(o_sorted, minlength=NSEC)
    W = (counts + 127) // 128                           # cols per section
    base = np.concatenate([[0], np.cumsum(W)])
    COLS = int(base[-1])

    F = np.zeros((N_CORES, 128, COLS * 2), np.float32)
    IDX = np.full((16, COLS * 8), -1, np.int16)
    sec_meta = []
    pos = 0
    for s in range(NSEC):
        cnt = int(counts[s])
        b0 = int(base[s])
        ii = np.arange(cnt)
        seg = slice(pos, pos + cnt)
        p = ii % 128
        c = b0 + ii // 128
        for core in range(N_CORES):
            F[core, p, c * 2] = sums_sorted[2 * core, seg]
            F[core, p, c * 2 + 1] = sums_sorted[2 * core + 1, seg]
        IDX[ii % 16, b0 * 8 + ii // 16] = blk_sorted[seg]
        sec_meta.append((b0, int(W[s]), cnt))
        pos += cnt
    IDX_rep = np.tile(IDX, (8, 1))
    return F, IDX_rep, sec_meta, COLS


def _build(sec_meta, COLS, with_clock=True):
    from concourse import bass, bacc, mybir
    from concourse import library_config
    from contextlib import ExitStack

    NQ = 4
    nc = bacc.Bacc("TRN2", target_bir_lowering=False, debug=False,
                   num_devices=N_CORES, num_swdge_queues=NQ)
    DT = mybir.dt.float32
    A = mybir.AluOpType
    f_d = nc.dram_tensor("fsrc", [128, COLS * 2], DT, kind="ExternalInput")
    idx_d = nc.dram_tensor("idx16", [128, COLS * 8], mybir.dt.int16,
                           kind="ExternalInput")
    grid_d = nc.dram_tensor("grid", [NBLK, 128], DT, kind="ExternalOutput")
    out_d = nc.dram_tensor("out", [128, 4], DT, kind="ExternalOutput")
    cnt_d = nc.dram_tensor("cnt", [1, 4], mybir.dt.uint32, kind="ExternalOutput")

    CH = 2048                 # f32/partition per z-chunk
    NCHZ = 32768 // CH        # 16
    NCHX = 16                 # x-diff chunks: 8 y-values each (8*254=2032)
    NCHY = 16                 # y-diff chunks of 2032 (16*2032=32512)
    NSLOT = NCHX + NCHY + NCHZ

    with ExitStack() as st:
        block = st.enter_context(nc.Block())
        io = st.enter_context(nc.semaphore("io"))
        scat_done = st.enter_context(nc.semaphore("scatdone"))
        gload = st.enter_context(nc.semaphore("gload"))
        zload = st.enter_context(nc.semaphore("zload"))
        dsub = st.enter_context(nc.semaphore("dsub"))
        asq = st.enter_context(nc.semaphore("asq"))
        vsub = st.enter_context(nc.semaphore("vsub"))
        psd = st.enter_context(nc.semaphore("psd"))
        fin = st.enter_context(nc.semaphore("fin"))
        start_sem = st.enter_context(nc.semaphore("startc"))
        sems = [nc.alloc_semaphore(f"sq{q}_{i}") for q in range(NQ) for i in range(8)]

        FLG = st.enter_context(nc.sbuf_tensor("FLG", [1, 4], mybir.dt.uint32))
        # --- phase-1 tensors (freed before G is allocated; scatter DMA
        # completion is ordered before the G readback via scat_done) ---
        with (nc.sbuf_tensor("F", [128, COLS * 2], DT) as F,
              nc.sbuf_tensor("IX", [128, COLS * 8], mybir.dt.int16) as IX):

            @block.gpsimd
            def _(gp):
                gp.load_library(library_config.mlp)
                gp.memset(FLG[:], 0)
                gp.dma_start(F[:], f_d[:]).then_inc(io, 16)
                gp.dma_start(IX[:], idx_d[:]).then_inc(io, 16)
                gp.wait_ge(io, 32)
                gp.sem_inc(start_sem, 1)
                counts_per_sem = {}
                k = 0
                MAXW = 63   # <= 8064 idxs/call fits the 1024-desc SWDGE ring
                for s in range(NSEC):
                    b0, w, cnt = sec_meta[s]
                    off = 0
                    while cnt > 0:
                        wsub = min(w - off, MAXW)
                        csub = min(cnt, wsub * 128)
                        q = k % NQ
                        sem = sems[q * 8 + (k // NQ) % 8]
                        bb = b0 + off
                        in_ap = F[:, bb * 2:(bb + wsub) * 2].rearrange(
                            "p (t e) -> p t e", e=2)
                        gp.dma_scatter_add(
                            grid_d[:, 2 * s:2 * s + 2], in_ap,
                            IX[:, bb * 8:(bb + wsub) * 8],
                            wsub * 128, csub, 2, elem_step=128, queue_num=q,
                        ).then_inc(sem, 16)
                        counts_per_sem[sem.num] = counts_per_sem.get(sem.num, 0) + 1
                        k += 1
                        off += wsub
                        cnt -= csub
                for sem in sems:
                    c = counts_per_sem.get(sem.num)
                    if c:
                        gp.wait_ge(sem, 16 * c)
                gp.memset(FLG[0:1, 1:2], 1)
                gp.sem_inc(scat_done, 1)

            if with_clock:
                @block.tensor
                def _(te):
                    cntr = te.alloc_register("cntr")
                    flag = te.alloc_register("flagr")
                    nd = te.alloc_register("nd")
                    te.reg_mov(cntr, 0)
                    te.wait_ge(start_sem, 1)
                    # poll cells in phase order: 1=scatter, 2=gload, 0=final;
                    # identical loop body keeps the per-iter calibration valid
                    for cell, col in ((1, 1), (2, 2), (0, 0)):
                        te.reg_mov(nd, 1)
                        with te.While(lambda: nd):
                            te.reg_load(flag, FLG[0:1, cell:cell + 1])
                            te.reg_alu(nd, flag, 0, A.is_equal)
                            te.reg_alu(cntr, cntr, 1, A.add)
                        te.reg_save(cnt_d[0:1, col:col + 1], cntr)

        FLG_keep = FLG  # the vector engine sets it at the end

        # --- phase-2 tensors overlay the freed F/IX region ---
        if True:
            with (nc.sbuf_tensor("G", [128, 32768], DT) as G,
                  nc.sbuf_tensor("D0", [128, CH], DT) as D0,
                  nc.sbuf_tensor("D1", [128, CH], DT) as D1,
                  nc.sbuf_tensor("GZ0", [128, CH], DT) as GZ0,
                  nc.sbuf_tensor("GZ1", [128, CH], DT) as GZ1,
                  nc.sbuf_tensor("PA", [128, 2 * NSLOT], DT) as PA,
                  nc.sbuf_tensor("PS", [128, 2 * NSLOT], DT) as PS,
                  nc.sbuf_tensor("OT", [128, 4], DT) as OT):

                @block.sync
                def _(sy):
                    sy.wait_ge(scat_done, 1)
                    g_view = grid_d[:].rearrange("(z b) c -> z (b c)", z=128)
                    sy.dma_start(G[:], g_view).then_inc(gload, 16)
                    gz_view = grid_d[256:, :].rearrange(
                        "(z b) c -> z (b c)", z=127)
                    for c in range(NCHZ):
                        if c >= 2:
                            sy.wait_ge(vsub, NCHX + NCHY + c - 1)
                        buf = (GZ0 if c % 2 == 0 else GZ1)
                        sy.dma_start(buf[0:127, :],
                                     gz_view[:, c * CH:(c + 1) * CH]
                                     ).then_inc(zload, 16)
                    sy.wait_ge(fin, 1)
                    sy.dma_start(out_d[:], OT[:]).then_inc(io, 16)
                    sy.wait_ge(io, 48)

                @block.vector
                def _(ve):
                    ve.wait_ge(gload, 16)
                    ve.memset(FLG_keep[0:1, 2:3], 1)
                    ve.memset(PA[:], 0)
                    ve.memset(PS[:], 0)
                    slot = 0

                    def reduce_pair(dch_flat, p):
                        nonlocal slot
                        ve.wait_ge(vsub, slot + 1)
                        dr = dch_flat.rearrange("p (f r) -> p r f", r=2)
                        ve.tensor_reduce(PA[0:p, slot * 2: slot * 2 + 2], dr,
                                         axis=mybir.AxisListType.X, op=A.add,
                                         apply_absolute_value=True
                                         ).then_inc(dsub, 1)
                        ve.wait_ge(asq, slot + 1)
                        ve.tensor_reduce(PS[0:p, slot * 2: slot * 2 + 2], dr,
                                         axis=mybir.AxisListType.X, op=A.add
                                         ).then_inc(psd, 1)
                        slot += 1

                    # x-diff: 16 chunks of 8 y-values
                    for c in range(NCHX):
                        dbuf = D0 if slot % 2 == 0 else D1
                        y0 = c * 8
                        gy = G[:, y0 * 256:(y0 + 8) * 256].rearrange(
                            "p (y x r) -> p y x r", y=8, x=128)
                        dch = dbuf[:, 0:8 * 254].rearrange(
                            "p (y x r) -> p y x r", y=8, x=127)
                        ve.tensor_tensor(dch, gy[:, :, 1:128, :],
                                         gy[:, :, 0:127, :], op=A.subtract
                                         ).then_inc(vsub, 1)
                        reduce_pair(dbuf[:, 0:8 * 254], 128)
                    # y-diff: 16 chunks of 2032
                    for c in range(NCHY):
                        dbuf = D0 if slot % 2 == 0 else D1
                        lo = c * 2032
                        ve.tensor_tensor(dbuf[:, 0:2032],
                                         G[:, 256 + lo: 256 + lo + 2032],
                                         G[:, lo: lo + 2032], op=A.subtract
                                         ).then_inc(vsub, 1)
                        reduce_pair(dbuf[:, 0:2032], 128)
                    # z-diff: 16 chunks from GZ bufs
                    for c in range(NCHZ):
                        dbuf = D0 if slot % 2 == 0 else D1
                        gz = GZ0 if c % 2 == 0 else GZ1
                        ve.wait_ge(zload, 16 * (c + 1))
                        ve.tensor_tensor(dbuf[0:127, 0:CH], gz[0:127, :],
                                         G[0:127, c * CH:(c + 1) * CH],
                                         op=A.subtract).then_inc(vsub, 1)
                        reduce_pair(dbuf[0:127, 0:CH], 127)
                    # final: sum slots keeping r
                    ve.wait_ge(dsub, NSLOT)
                    ve.wait_ge(psd, NSLOT)
                    pa3 = PA[:].rearrange("p (s r) -> p r s", r=2)
                    ps3 = PS[:].rearrange("p (s r) -> p r s", r=2)
                    ve.tensor_reduce(OT[:, 0:2], pa3,
                                     axis=mybir.AxisListType.X, op=A.add)
                    ve.tensor_reduce(OT[:, 2:4], ps3,
                                     axis=mybir.AxisListType.X, op=A.add
                                     ).then_inc(fin, 1)
                    ve.memset(FLG_keep[:], 1)

                @block.scalar
                def _(sc):
                    for slot in range(NSLOT):
                        sc.wait_ge(dsub, slot + 1)
                        dbuf = D0 if slot % 2 == 0 else D1
                        if slot < NCHX:
                            wid, p = 8 * 254, 128
                        elif slot < NCHX + NCHY:
                            wid, p = 2032, 128
                        else:
                            wid, p = CH, 127
                        sc.activation(dbuf[0:p, 0:wid], dbuf[0:p, 0:wid],
                                      mybir.ActivationFunctionType.Square
                                      ).then_inc(asq, 1)

    nc.compile()
    return nc


def kernel(values, indices, xsize):
    from concourse.bass_utils import run_bass_kernel_spmd
    values = np.asarray(values, np.float32)
    indices = np.asarray(indices)
    F, IDX_rep, sec_meta, COLS = _host_prep(values, indices)
    key = tuple(map(tuple, sec_meta)) + (COLS,)
    if key not in _compiled:
        _compiled[key] = _build(sec_meta, COLS)
    nc = _compiled[key]
    in_maps = [{"fsrc": F[c], "idx16": IDX_rep} for c in range(N_CORES)]
    res = run_bass_kernel_spmd(nc, in_maps, list(range(N_CORES)))
    tv = np.zeros(B, np.float32)
    mse = np.zeros(B, np.float32)
    clk = []
    phases = []
    for c in range(N_CORES):
        part = res.results[c]["out"]          # [128, 4]
        tots = part.sum(axis=0, dtype=np.float64)
        tv[2 * c] = tots[0] / V
        tv[2 * c + 1] = tots[1] / V
        mse[2 * c] = tots[2] / NUM_PAIRS
        mse[2 * c + 1] = tots[3] / NUM_PAIRS
        cnt = res.results[c]["cnt"]
        clk.append(int(cnt[0, 0]))
        phases.append((int(cnt[0, 1]), int(cnt[0, 2]), int(cnt[0, 0])))
    kernel.last_clock_iters = clk
    kernel.last_clock_phases = phases
    return tv, mse

